# revision 33
# baseline (speedup 1.0000x reference)
"""Multi-head self-attention (b=2, n=2048, emb=1024, heads=16) on 8 trn2 cores.

Sharding: core c = (b, hg) with b = c // 4, hg = c % 4. Data parallel over
batch, tensor parallel over head-groups (4 heads / 256 emb-cols per core).
Each core computes Q/K/V projections for its heads, full attention for its
heads, and a partial output projection ctx_hg @ Wo[:, hg_slice].T of shape
[2048, 1024]. The host sums the 4 partials per batch (Megatron row-parallel
reduce done on host) and adds the rank-1 bias term bv @ Wo.T + bo.

The kernel is built as ONE flat software-pipelined stream over all
(head-pair, nq-window, nk-group) attention work items, with projections and
the output projection woven in as filler parcels. Design points, found by
perfetto-trace iteration (ACT's 128 1024-wide exps = 142.5us are the
engine floor; everything else hides behind them):

- S matmuls are K=64: the two heads of a pair sit in disjoint PE row
  groups (kT/qT rows 0:64 / 64:128), issued as adjacent instruction pairs
  so the array runs them concurrently (row tiling, ~2x S throughput). At
  each j-window's first group the heads are issued as blocks instead
  (h0's s-tile frees one exp earlier than h1's -- pairing would
  head-of-line block on the s1 WAR and bubble the exp stream).
- ctx matmul: stationary v blocks are 128 wide: col 0 = ones (softmax row
  sums land on ctx partition 0 = the gpsimd partition_broadcast source),
  cols 64:128 = v (ctx rows at base-64; DVE ops need 32-aligned bases).
- softmax normalize: rowsum broadcast (gpsimd) -> reciprocal_approx_fast
  (custom DVE op, ~5x faster than the iterative-divide reciprocal, ~51
  ULP) -> one multiply into fp16 ctxT. Runs off the critical path; the
  ctx PSUM bank is released by two small copies first.
- out-projection: per (m, eo) parcel both head-pair chunks accumulate in
  one PSUM group -> one DVE copy -> one store. Parcels are scheduled as
  fillers right after their ctxT1 columns finalize; the last two
  m-chunks' kp0 halves run early so the post-loop tail is only kp1.
- x loads: one DMA per (k, n-pair) with 2KB-contiguous per-partition rows
  (xT rows are 4KB in HBM); weights host-pretiled to 4KB rows, one DMA
  each. DMA triggers cost ~600ns of the issuing engine's sequencer, so
  the scalar-engine DGE queue is used only while ACT is idle (startup x
  chunk 0 / wv / wq) and for the tail stores; all else rides qSync.
- K/V/Q parcels are emitted just-in-time inside the early j-windows of
  their pair (work item (j, g) only reads kT cols of group g, v of group
  g, qT of window j) -- program order must respect def-before-use, which
  the dependency tracker does NOT enforce for these access patterns.
- All matmuls run in float16 (1 cyc/col on PE + fast weight load; the
  overall error is ~7e-4 scale-relative, validated vs fp32). q/k biases
  are added on-device (fused into the PSUM->SBUF copy); v/o biases are
  exactly the rank-1 host-side term above.
- PE idle gaps >3.4us re-throttle the PE clock to 1.2GHz (HAM); the
  filler pacing (mid-window pops only) keeps it at 2.4GHz throughout.
"""

import os
import sys

for _p in ("/opt/trn_rl_repo", "/root/.axon_site/_ro/trn_rl_repo"):
    if os.path.isdir(_p) and _p not in sys.path:
        sys.path.append(_p)

import numpy as np

import concourse.bass as bass  # noqa: F401  (engine types pulled via nc)
import concourse.mybir as mybir
import concourse.tile as tile
from concourse import bacc
from concourse.bass_utils import run_bass_kernel_spmd

B, N, EMB, HEADS, HD = 2, 2048, 1024, 16, 64
N_CORES = 8
TP = 4                      # head-group shards per batch
DQ = EMB // TP              # 256 emb-cols (4 heads) per core
SCALE = HD ** -0.5          # 0.125

F32 = mybir.dt.float32
F16 = mybir.dt.float16
FP = mybir.ActivationFunctionType

NQ = 512                    # nq chunk for projections / out-proj (moving free dim)
NJ = N // NQ                # 4 nq chunks
NQA = 256                   # nq chunk for attention (so 4 nk-chunks fit one exp)
NJA = N // NQA              # 8 attention nq chunks
NKC = 128                   # nk chunk (ctx contraction)
NT = N // NKC               # 16 nk chunks
KC = EMB // 128             # 8 e chunks
# nk-chunk groups per exp instruction (4 x 256 -> 1024-wide exps).
# PSUM budget (8 banks): pp 2 + s0 2 + s1 2 + c0 1 + c1 1.
T_GROUPS = [tuple(range(0, 4)), tuple(range(4, 8)), tuple(range(8, 12)),
            tuple(range(12, 16))]


def build_program():
    """Build + compile the single SPMD program all 8 cores run."""
    nc = bacc.Bacc("TRN2", target_bir_lowering=False, debug=False,
                   num_devices=N_CORES)

    xT = nc.dram_tensor("xT", [EMB, N], F16, kind="ExternalInput").ap()
    wqT = nc.dram_tensor("wqT", [128, KC * DQ], F16, kind="ExternalInput").ap()
    wkT = nc.dram_tensor("wkT", [128, KC * DQ], F16, kind="ExternalInput").ap()
    wvT = nc.dram_tensor("wvT", [128, KC * DQ], F16, kind="ExternalInput").ap()
    woT = nc.dram_tensor("woT", [128, 2 * EMB], F16, kind="ExternalInput").ap()
    bqd = nc.dram_tensor("bq_s", [DQ], F32, kind="ExternalInput").ap()
    bkd = nc.dram_tensor("bk_s", [DQ], F32, kind="ExternalInput").ap()
    out_part = nc.dram_tensor("out_part", [N, EMB], F32,
                              kind="ExternalOutput").ap()

    with tile.TileContext(nc) as tc:
        with (
            tc.tile_pool(name="const", bufs=1) as const,
            tc.tile_pool(name="xp", bufs=24) as xp,
            tc.tile_pool(name="persist", bufs=1) as persist,
            tc.tile_pool(name="epool", bufs=3) as epool,
            tc.tile_pool(name="npool", bufs=2) as npool,
            tc.tile_pool(name="opool", bufs=4) as opool,
            # PSUM static budget (8 banks): pp 2 + s0 2 + s1 2 + c0 1 + c1 1
            tc.tile_pool(name="ppool", bufs=2, space="PSUM") as ppool,
            tc.tile_pool(name="spool", bufs=1, space="PSUM") as spool,
            tc.tile_pool(name="cpool", bufs=1, space="PSUM") as cpool,
        ):
            # ---- constants ----
            # weights arrive host-pretiled: one DMA each, 4KB-contiguous
            # per-partition rows (big DMA packets, low descriptor count)
            wk_sb = const.tile([128, KC, DQ], F16, tag="wk")
            nc.sync.dma_start(out=wk_sb, in_=wkT)
            wv_sb = const.tile([128, KC, DQ], F16, tag="wv")
            nc.scalar.dma_start(out=wv_sb, in_=wvT)
            # wq's DMA is issued after chunk 0 of x is queued on this DGE
            # queue -- K(0,0) and the first S group are gated on x chunk 0,
            # not on wq
            wq_sb = const.tile([128, KC, DQ], F16, tag="wq")
            # wo is needed only by the out-projection (~60us in) -- its DMA
            # is deferred into the filler stream to keep startup queues clear
            wo_sb = const.tile([128, 2, EMB], F16, tag="wo")
            bq_sb = const.tile([128, 2], F32, tag="bq")
            nc.sync.dma_start(out=bq_sb, in_=bqd.rearrange("(m p) -> p m", p=128))
            bk_sb = const.tile([128, 2], F32, tag="bk")
            nc.sync.dma_start(out=bk_sb, in_=bkd.rearrange("(m p) -> p m", p=128))

            # ---- persistent activations ----
            qT = [persist.tile([128, N], F16, tag=f"qT{p}", name=f"qT{p}") for p in range(2)]
            kT = [persist.tile([128, N], F16, tag=f"kT{p}", name=f"kT{p}") for p in range(2)]
            ctxT = [persist.tile([128, N], F16, tag=f"ctxT{p}", name=f"ctxT{p}") for p in range(2)]
            # V for all 4 local heads, 128-wide stationary blocks:
            # col 0 = ones (rowsum -> ctx partition 0, the broadcast source),
            # cols 64:128 = v (ctx rows at base-64). Cols 1:64 are unread
            # pad, zeroed once for the simulator.
            v_all = persist.tile([128, NT, 4 * 128], F16, tag="v")
            for h in range(4):
                nc.vector.memset(v_all[:, :, h * 128 + 1:h * 128 + 64], 0.0)
                nc.vector.memset(v_all[:, :, h * 128], 1.0)

            add, mult = mybir.AluOpType.add, mybir.AluOpType.mult

            # ---- projection building blocks ----
            # Each returns/consumes one PSUM accumulation group, small enough
            # to slot between attention groups without starving ACT.
            _xts = {}

            def load_x_pair(p, n):
                # one DMA per k covering TWO n-chunks: per-partition rows are
                # 2KB contiguous in HBM (xT rows are 4KB), halving descriptor
                # count and roughly doubling queue throughput. Queue choice:
                # pair-0 loads run while ACT is idle, so they may use the
                # scalar-engine DGE queue; everything later stays off ACT
                # (each DMA trigger costs ~600ns of its engine's sequencer).
                xts = []
                for k in range(KC):
                    xt = xp.tile([128, 2 * NQ], F16, tag="xt", name="xt")
                    eng = nc.scalar if (p == 0 and k % 2 == 1) else nc.sync
                    eng.dma_start(
                        out=xt,
                        in_=xT[k * 128:(k + 1) * 128,
                               n * NQ:(n + 2) * NQ])
                    xts.append(xt)
                _xts[(p, n)] = [t[:, 0:NQ] for t in xts]
                _xts[(p, n + 1)] = [t[:, NQ:2 * NQ] for t in xts]

            def load_x_chunk(pn):
                p, n = pn
                if (p, n) in _xts:
                    return
                load_x_pair(p, n if n % 2 == 0 else n - 1)

            _kq_ps = {}

            def kq_group_a(p, n, wsb):
                xts = _xts[(p, n)]
                ps = ppool.tile([128, NQ], F32, tag="pp", name="kqp")
                _kq_ps[(p, n)] = ps
                for k in range(KC // 2):
                    nc.tensor.matmul(
                        ps, wsb[:, k, p * 128:(p + 1) * 128],
                        xts[k], start=(k == 0), stop=False)

            def kq_group_b(p, n, wsb, bsb, dst):
                xts = _xts[(p, n)]
                ps = _kq_ps.pop((p, n))
                for k in range(KC // 2, KC):
                    nc.tensor.matmul(
                        ps, wsb[:, k, p * 128:(p + 1) * 128],
                        xts[k], start=False, stop=(k == KC - 1))
                nc.vector.tensor_tensor(
                    out=dst[p][:, n * NQ:(n + 1) * NQ], in0=ps,
                    in1=bsb[:, p:p + 1].broadcast_to([128, NQ]), op=add)

            def kq_group(p, n, wsb, bsb, dst):
                kq_group_a(p, n, wsb)
                kq_group_b(p, n, wsb, bsb, dst)

            def v_group(p, n, tl):
                xts = _xts[(p, n)]
                t = n * 4 + tl
                ps = ppool.tile([128, NQ], F32, tag="pp", name="vp")
                for k in range(KC):
                    nc.tensor.matmul(
                        ps[:, 0:128], xts[k][:, tl * 128:(tl + 1) * 128],
                        wv_sb[:, k, p * 128:(p + 1) * 128],
                        start=(k == 0), stop=(k == KC - 1))
                vv = v_all[:, t, :].rearrange("p (h c) -> p h c", c=128)
                nc.vector.tensor_copy(
                    out=vv[:, 2 * p:2 * p + 2, 64:128],
                    in_=ps[:, 0:128].rearrange("p (h c) -> p h c", c=64))

            def proj_fillers(p):
                # per n-chunk: K + 4 V + Q as 6 filler parcels; the x-chunk
                # DMAs are issued one n-chunk ahead so PE never head-of-line
                # blocks on a fresh load
                out = [lambda p=p: load_x_chunk((p, 0)),
                       lambda p=p: load_x_chunk((p, 1))]
                for n in range(NJ):
                    out.append(lambda p=p, n=n: kq_group_a(p, n, wk_sb))
                    out.append(lambda p=p, n=n: kq_group_b(p, n, wk_sb, bk_sb, kT))
                    for tl in range(4):
                        out.append(lambda p=p, n=n, tl=tl: v_group(p, n, tl))
                    out.append(lambda p=p, n=n: kq_group_a(p, n, wq_sb))
                    out.append(lambda p=p, n=n: (
                        kq_group_b(p, n, wq_sb, bq_sb, qT),
                        _xts.pop((p, n))))
                    if n + 2 < NJ:
                        out.insert(-6, lambda p=p, n=n: load_x_chunk((p, n + 2)))
                return out

            # pair-0: only chunk 0 runs before the attention loop. The
            # remaining K/V/Q parcels are emitted just-in-time at forced
            # positions inside the first j windows (attention work item
            # (j, g) only reads kT cols of group g, v chunks of group g,
            # and qT cols of window j -- def must precede use in program
            # order, the scheduler handles the rest).
            # HAM warm-up: the x drip-feed keeps PE under the 3.4us
            # sustained-busy threshold, so the whole prefix would run at the
            # cold 1.2GHz clock. ~3us of tiny matmuls on the already-landed
            # bias tiles warms it to 2.4GHz right as real data arrives.
            warm0 = spool.tile([128, 4, NQA], F32, tag="s0", name="warm0")
            for _ in range(30):
                nc.tensor.matmul(warm0[0:2, 0, 0:2], bq_sb, bk_sb,
                                 start=True, stop=True)
            load_x_pair(0, 0)
            nc.sync.dma_start(out=wq_sb, in_=wqT)
            load_x_pair(0, 2)
            kq_group_a(0, 0, wk_sb)
            kq_group_b(0, 0, wk_sb, bk_sb, kT)
            for tl in range(4):
                v_group(0, 0, tl)
            kq_group_a(0, 0, wq_sb)
            kq_group_b(0, 0, wq_sb, bq_sb, qT)

            def _q0(n):
                kq_group_a(0, n, wq_sb)
                kq_group_b(0, n, wq_sb, bq_sb, qT)
                _xts.pop((0, n))

            def _q(p, n):
                kq_group_a(p, n, wq_sb)
                kq_group_b(p, n, wq_sb, bq_sb, qT)
                _xts.pop((p, n), None)

            def _jit(p):
                # just-in-time K/V/Q emission map for head-pair p's window:
                # chunk n's K before S reads kT cols of group n, V(n) before
                # ctx of group n, Q(n) before the j window that reads it.
                return {
                    (p, 0, 1): [lambda: kq_group_a(p, 1, wk_sb),
                                lambda: kq_group_b(p, 1, wk_sb, bk_sb, kT)],
                    (p, 0, 2): [lambda: kq_group_a(p, 2, wk_sb),
                                lambda: kq_group_b(p, 2, wk_sb, bk_sb, kT)]
                               + [lambda tl=tl: v_group(p, 1, tl)
                                  for tl in range(4)],
                    (p, 0, 3): [lambda: kq_group_a(p, 3, wk_sb),
                                lambda: kq_group_b(p, 3, wk_sb, bk_sb, kT)]
                               + [lambda tl=tl: v_group(p, 2, tl)
                                  for tl in range(4)],
                    (p, 1, 0): [lambda tl=tl: v_group(p, 3, tl)
                                for tl in range(4)],
                    (p, 1, 1): [lambda: _q(p, 1)],
                    (p, 2, 0): [lambda: _q(p, 2)],
                    (p, 3, 0): [lambda: _q(p, 3),
                                lambda: _xts.pop((p, 0), None)],
                }

            pre_items = _jit(0)

            # ---- attention (per head-pair p, nq chunk j of 256) ----
            # Per t-chunk the two heads' S matmuls (K=64, disjoint PE
            # row-groups 0:64 / 64:128) are issued back-to-back so the array
            # runs them concurrently. Software-pipelined: ctx matmuls for
            # group g are emitted after the S/exp of group g+1, so PE always
            # has ready work while ACT streams wide exps. The ctx PSUM bank
            # is released by one quick copy to SBUF; the reciprocal-normalize
            # then runs off the critical path on DVE/GpSimd.
            _tail = [False]
            _po_open = {}

            def out_proj_a(m, eo):
                # kp=0 half: reads only ctxT0 (ready well before the final
                # j's normalize) -- emitted early for the last m-chunks so
                # the post-normalize tail chain is just the kp=1 half.
                po = ppool.tile([128, NQ], F32, tag="pp", name="po")
                _po_open[(m, eo)] = po
                nc.tensor.matmul(
                    po, ctxT[0][:, m * 128:(m + 1) * 128],
                    wo_sb[:, 0, eo * NQ:(eo + 1) * NQ],
                    start=True, stop=False)

            def out_proj_b(m, eo):
                po = _po_open.pop((m, eo))
                nc.tensor.matmul(
                    po, ctxT[1][:, m * 128:(m + 1) * 128],
                    wo_sb[:, 1, eo * NQ:(eo + 1) * NQ],
                    start=False, stop=True)
                o = opool.tile([128, NQ], F32, tag="o", name="o")
                nc.vector.tensor_copy(o, po)
                eng = nc.scalar if (_tail[0] and eo == 1) else nc.sync
                eng.dma_start(
                    out=out_part[m * 128:(m + 1) * 128,
                                 eo * NQ:(eo + 1) * NQ], in_=o)

            def out_proj_parcel(m, eo):
                # one (m, eo) parcel: both kp chunks accumulate in a single
                # PSUM group, one DVE copy, one store.
                out_proj_a(m, eo)
                out_proj_b(m, eo)

            from collections import deque
            fillers = deque()

            # Flat software-pipelined stream over all (p, j, group) work
            # items. ctx for item i is emitted after S/exp of item i+1 --
            # ACROSS j and p boundaries -- so the next j's S matmuls are
            # never queued behind the previous j's ctx drain + fillers
            # (in-order PE queue). The normalize for a j is emitted right
            # after its last ctx group, i.e. inside the next j's first
            # work item.
            def s_mms_pair(p, j, g, split_heads=False):
                sps = [spool.tile([128, len(g), NQA], F32,
                                  tag=f"s{h}", name=f"s{h}")
                       for h in range(2)]
                # split_heads: h0's block first so it can run during the
                # previous group's h1 exp (s1-tag WAR would head-of-line
                # block an interleaved pair stream); costs g0's row-tile
                # concurrency, which PE has slack for at the j boundary.
                order = ([(i, t, h) for h in range(2) for i, t in enumerate(g)]
                         if split_heads else
                         [(i, t, h) for i, t in enumerate(g) for h in range(2)])
                for i, t, h in order:
                    lo = 64 * h
                    nc.tensor.matmul(
                        sps[h][:, i, :],
                        kT[p][lo:lo + 64, t * 128:(t + 1) * 128],
                        qT[p][lo:lo + 64, j * NQA:(j + 1) * NQA],
                        start=True, stop=True)
                return sps

            def exp_act(sp, g, h):
                e = epool.tile([128, len(g), NQA], F16,
                               tag=f"e{h}", name=f"e{h}")
                nc.scalar.activation(e, sp, FP.Exp, scale=SCALE)
                return e

            def ctx_mms_pair(es, g, p, cps):
                for h in range(2):
                    hloc = 2 * p + h
                    for i, t in enumerate(g):
                        nc.tensor.matmul(
                            cps[h],
                            v_all[:, t, hloc * 128:(hloc + 1) * 128],
                            es[h][:, i, :],
                            start=(t == 0), stop=(t == NT - 1))

            def do_normalize(p, j, cps, final=False):
                css = []
                for h in range(2):
                    cs = npool.tile([128, NQA], F32, tag=f"cs{h}",
                                    name=f"cs{h}")
                    nc.vector.tensor_copy(cs[0:1, :], cps[h][0:1, :])
                    if not final:
                        # frees the ctx PSUM bank for the next j; for the
                        # final window the mult reads PSUM directly instead
                        # (the bank is not reused, and the copy would sit in
                        # the serial tail chain)
                        nc.vector.tensor_copy(cs[64:128, :], cps[h][64:128, :])
                    css.append(cs)
                for h in range(2):
                    cs = css[h]
                    rb = npool.tile([128, NQA], F32, tag=f"rb{h}",
                                    name=f"rb{h}")
                    nc.gpsimd.partition_broadcast(rb, cs[0:1, :])
                    rc = npool.tile([128, NQA], F32, tag=f"rc{h}",
                                    name=f"rc{h}")
                    nc.vector.reciprocal_approx_fast(out=rc, in_=rb)
                    src_rows = cps[h][64:128, :] if final else cs[64:128, :]
                    nc.vector.tensor_tensor(
                        out=ctxT[p][h * 64:(h + 1) * 64,
                                    j * NQA:(j + 1) * NQA],
                        in0=src_rows, in1=rc[64:128, :], op=mult)
                if p == 1:
                    # ctxT1 columns for this j are final -> out-proj
                    # parcels for the covered m-chunks can run
                    if j == NJA - 2:
                        # next j is the last: presplit its parcels -- the
                        # kp0 halves (ctxT0-only) go in NOW as fillers, the
                        # kp1 halves run in the post-normalize drain
                        for m in (2 * j, 2 * j + 1):
                            for eo in range(2):
                                fillers.append(
                                    lambda m=m, eo=eo: out_proj_parcel(m, eo))
                        for m in (2 * j + 2, 2 * j + 3):
                            for eo in range(2):
                                fillers.append(
                                    lambda m=m, eo=eo: out_proj_a(m, eo))
                    elif j == NJA - 1:
                        for m in (2 * j, 2 * j + 1):
                            for eo in range(2):
                                fillers.append(
                                    lambda m=m, eo=eo: out_proj_b(m, eo))
                    else:
                        for m in (2 * j, 2 * j + 1):
                            for eo in range(2):
                                fillers.append(
                                    lambda m=m, eo=eo: out_proj_parcel(m, eo))

            _tail = [False]
            _po_open = {}

            def out_proj_a(m, eo):
                # kp=0 half: reads only ctxT0 (ready well before the final
                # j's normalize) -- emitted early for the last m-chunks so
                # the post-normalize tail chain is just the kp=1 half.
                po = ppool.tile([128, NQ], F32, tag="pp", name="po")
                _po_open[(m, eo)] = po
                nc.tensor.matmul(
                    po, ctxT[0][:, m * 128:(m + 1) * 128],
                    wo_sb[:, 0, eo * NQ:(eo + 1) * NQ],
                    start=True, stop=False)

            def out_proj_b(m, eo):
                po = _po_open.pop((m, eo))
                nc.tensor.matmul(
                    po, ctxT[1][:, m * 128:(m + 1) * 128],
                    wo_sb[:, 1, eo * NQ:(eo + 1) * NQ],
                    start=False, stop=True)
                o = opool.tile([128, NQ], F32, tag="o", name="o")
                nc.vector.tensor_copy(o, po)
                eng = nc.scalar if (_tail[0] and eo == 1) else nc.sync
                eng.dma_start(
                    out=out_part[m * 128:(m + 1) * 128,
                                 eo * NQ:(eo + 1) * NQ], in_=o)

            def out_proj_parcel(m, eo):
                # one (m, eo) parcel: both kp chunks accumulate in a single
                # PSUM group, one DVE copy, one store.
                out_proj_a(m, eo)
                out_proj_b(m, eo)

            fillers.append(lambda: nc.sync.dma_start(out=wo_sb, in_=woT))
            fillers.append(lambda: load_x_pair(1, 0))
            fillers.append(lambda: load_x_pair(1, 2))
            for n in range(NJ):
                fillers.append(lambda n=n: kq_group_a(1, n, wk_sb))
                fillers.append(lambda n=n: kq_group_b(1, n, wk_sb, bk_sb, kT))
                fillers.extend(
                    (lambda n=n, tl=tl: v_group(1, n, tl)) for tl in range(4))
                fillers.append(lambda n=n: kq_group_a(1, n, wq_sb))
                fillers.append(lambda n=n: (
                    kq_group_b(1, n, wq_sb, bq_sb, qT),
                    _xts.pop((1, n), None)))

            prev = None  # (es, g, p, cps, is_last_group, j)
            for p in range(2):
                for j in range(NJA):
                    # pace: spread this window's fillers over its remaining
                    # j-iterations (pair-1 projections MUST drain within
                    # pair-0's window -- p1's S matmuls consume qT1/kT1).
                    # p0's j0 is fully packed with just-in-time parcels.
                    if j == 0:
                        n_pop = 0
                    else:
                        n_pop = -(-len(fillers) // (NJA - j))
                    cps = None
                    for wi, g in enumerate(T_GROUPS):
                        for f in pre_items.pop((p, j, wi), ()):
                            f()
                        sps = s_mms_pair(p, j, g, split_heads=(wi == 0))
                        es = [exp_act(sps[h], g, h) for h in range(2)]
                        if prev is not None:
                            pes, pg, pp_, pcps, plast, pj = prev
                            ctx_mms_pair(pes, pg, pp_, pcps)
                            if plast:
                                do_normalize(pp_, pj, pcps)
                        if cps is None:
                            # allocate AFTER the previous j's last ctx +
                            # normalize are emitted: the pool snapshots the
                            # old tile's uses at alloc time, so an earlier
                            # alloc would miss them and race
                            cps = [cpool.tile([128, NQA], F32, tag=f"c{h}",
                                              name=f"c{h}")
                                   for h in range(2)]
                        prev = (es, g, p, cps, wi == len(T_GROUPS) - 1, j)
                        # pop fillers only at mid-window slots: a pop at the
                        # j boundary wedges ahead of the next j's S matmuls
                        # in the in-order PE queue and stalls the exp stream
                        if wi > 0:
                            for _ in range(-(-n_pop // 3)):
                                if fillers and n_pop > 0:
                                    fillers.popleft()()
                                    n_pop -= 1
            pes, pg, pp_, pcps, plast, pj = prev
            ctx_mms_pair(pes, pg, pp_, pcps)
            _tail[0] = True
            # keep the PE clock warm through the normalize latency (a >3.4us
            # idle window would halve the final out-proj matmuls' rate):
            # a few tiny matmuls into the now-free s0 bank
            warm = spool.tile([128, 4, NQA], F32, tag="s0", name="warm")
            for w in range(4):
                nc.tensor.matmul(
                    warm[:, 0, 0:16], kT[1][0:64, 0:128],
                    qT[1][0:64, 0:16], start=True, stop=True)
            do_normalize(pp_, pj, pcps, final=True)
            while fillers:
                fillers.popleft()()

    nc.compile()
    return nc


_NC_CACHE = {}


def _get_program():
    if "nc" not in _NC_CACHE:
        _NC_CACHE["nc"] = build_program()
    return _NC_CACHE["nc"]


def _tile_w(w):
    # [KC*128, DQ] -> [128, KC*DQ] with per-partition rows k-major
    return np.ascontiguousarray(
        w.reshape(KC, 128, DQ).transpose(1, 0, 2).reshape(128, KC * DQ))


def _tile_wo(w):
    # [2*128, EMB] -> [128, 2*EMB]
    return np.ascontiguousarray(
        w.reshape(2, 128, EMB).transpose(1, 0, 2).reshape(128, 2 * EMB))


def make_in_maps(x, Wq, bq, Wk, bk, Wv, bv, Wo, bo):
    x = np.asarray(x)
    xTs = [np.ascontiguousarray(x[b].T.astype(np.float16)) for b in range(B)]
    in_maps = []
    for c in range(N_CORES):
        b, hg = divmod(c, TP)
        sl = slice(hg * DQ, (hg + 1) * DQ)
        in_maps.append({
            "xT": xTs[b],
            "wqT": _tile_w(np.asarray(Wq, np.float16)[sl, :].T),
            "wkT": _tile_w(np.asarray(Wk, np.float16)[sl, :].T),
            "wvT": _tile_w(np.asarray(Wv, np.float16)[sl, :].T),
            "woT": _tile_wo(np.asarray(Wo, np.float16)[:, sl].T),
            "bq_s": np.ascontiguousarray(np.asarray(bq, np.float32)[sl]),
            "bk_s": np.ascontiguousarray(np.asarray(bk, np.float32)[sl]),
        })
    return in_maps


def assemble_output(results, Wv_bias_term):
    out = np.empty((B, N, EMB), np.float32)
    for b in range(B):
        acc = results[b * TP]["out_part"].astype(np.float32)
        for g in range(1, TP):
            acc = acc + results[b * TP + g]["out_part"]
        out[b] = acc + Wv_bias_term
    return out


def kernel(x, Wq, bq, Wk, bk, Wv, bv, Wo, bo):
    nc = _get_program()
    in_maps = make_in_maps(x, Wq, bq, Wk, bk, Wv, bv, Wo, bo)
    res = run_bass_kernel_spmd(nc, in_maps, list(range(N_CORES)))
    bias_term = (np.asarray(bv, np.float32) @ np.asarray(Wo, np.float32).T
                 + np.asarray(bo, np.float32))
    return assemble_output(res.results, bias_term)
